# revision 19
# baseline (speedup 1.0000x reference)
"""LoRA-injected 3x3 conv (MoE-routed adapters), Trainium2 Bass kernel.

v3: 1D Winograd F(2,3) along H on top of the v2 bf16 kernel.

 - Host: merge each sample's LoRA adapter into the base conv weight
   (W_eff = conv_w + scale*active * up @ down -- exact low-rank merge),
   then Winograd-transform over kh: Gg = (g0, (g0+g1+g2)/2,
   (g0-g1+g2)/2, g2), pre-transpose to lhsT layout [ci, pos, kw, co],
   cast to bf16, shard batch across 8 cores (2 samples each).
 - Device: output rows processed in pairs (16 row-tiles per 32-row
   half). Input transform on GpSimd builds M_i = (d0-d2, d1+d2, d2-d1,
   d1-d3) over row 4-windows; the conv then needs only kw-taps: K per
   Winograd position = 320ci * 3kw = 960 = 7.5 x 128 accumulation
   steps instead of 22.5 for direct conv (1.5x less PE work). The ci
   tail chunk (64) packs kw0+kw1 into one K=128 step via a +1-column-
   shifted copy in partitions 64:127; the kw2 K=64 step is row-tiled:
   two Winograd positions' steps run concurrently in the two row
   halves of the PE array.
 - Cout=320 splits 128+128+64; the 64-row chunk is col-tiled (both
   r-groups concurrently in the two column halves of the array).
 - Output transform on DVE straight out of PSUM (GpSimd has no PSUM
   port on trn2) with scalar_tensor_tensor: y_even = (Y1+Y2)+bias+Y0,
   y_odd = (Y1-Y2)+bias-Y3, parities written interleaved into one
   staging tile so stores are single contiguous transfers on the
   scalar HWDGE queue. Weight loads are hoisted ahead of all stores.
"""

import sys

for _p in ("/opt/trn_rl_repo",):
    if _p not in sys.path:
        sys.path.insert(0, _p)

import numpy as np
import ml_dtypes

B, CIN, COUT, H, W = 16, 320, 320, 64, 64
R, NUM_LORAS, LORA_STRIDE, SCALE = 4, 50, 4, 1.0
NCORES = 8
BLOC = B // NCORES          # samples per core
HP, WP = H + 2, W + 2       # padded spatial
SP = HP * WP                # padded flat spatial per channel
HWFLAT = H * W
HHALF = H // 2              # rows per half-image
HALF_IN = (HHALF + 2) * WP  # 2244 padded elements per half
HALF_BASE = HHALF * WP      # 2112 flat offset of second half
NPOS = 4                    # Winograd F(2,3) positions
RT = 16                     # row-tiles (output row pairs) per half
MSP = RT * WP               # 1056 elements per M-tile partition
SPC = 512                   # psum chain width: 8 row-tiles x 64 cols
CO_CHUNKS = [(0, 128), (128, 128), (256, 64)]

_NC_CACHE = {}


def _build_nc():
    import concourse.bacc as bacc
    import concourse.bass as bass
    import concourse.mybir as mybir
    from concourse import tile

    f32 = mybir.dt.float32
    bf16 = mybir.dt.bfloat16

    nc = bacc.Bacc(None, target_bir_lowering=False)

    xp_d = nc.dram_tensor("xp", [BLOC, CIN, SP], bf16, kind="ExternalInput")
    # [b, ci(256), pos(4) x kw(3) x co]
    wt_d = nc.dram_tensor("wt", [BLOC, 256, 12 * COUT], bf16, kind="ExternalInput")
    # [b, 64:kw0|64:kw1 of tail ci, pos(4) x co]
    wp_d = nc.dram_tensor("wp", [BLOC, 128, 4 * COUT], bf16, kind="ExternalInput")
    # [b, kw2 tail ci duplicated in both halves, pos(4) x co]
    wq_d = nc.dram_tensor("wq", [BLOC, 128, 4 * COUT], bf16, kind="ExternalInput")
    bias_d = nc.dram_tensor("bias", [128, 3], f32, kind="ExternalInput")
    y_d = nc.dram_tensor("y", [BLOC, COUT, HWFLAT], bf16, kind="ExternalOutput")

    with tile.TileContext(nc) as tc:
        with (
            tc.tile_pool(name="io", bufs=1) as io_pool,
            tc.tile_pool(name="xin", bufs=2) as xpool,
            tc.tile_pool(name="mtr", bufs=2) as mpool,
            tc.tile_pool(name="const", bufs=1) as cpool,
            tc.tile_pool(name="ostage", bufs=4) as opool,
            tc.tile_pool(name="acc", bufs=8, space=bass.MemorySpace.PSUM) as pspool,
        ):
            bias_t = cpool.tile([128, 3], f32, tag="bias")

            def emit_drain(ps, cc, och0, ncha, b, stores):
                """Output transform + bias from PSUM. DVE has one PSUM read
                port, so ACT first lifts Y1+bias to SBUF; each DVE op then
                reads at most one PSUM operand. Parities interleave into one
                staging tile so each store is one contiguous transfer."""
                y0, y1, y2, y3 = (p.rearrange("p (r w) -> p r w", w=W) for p in ps)
                c1 = opool.tile([128, SPC], f32, tag="c1")
                t1 = opool.tile([128, SPC], f32, tag="t1")
                t2 = opool.tile([128, SPC], f32, tag="t2")
                oi = opool.tile([128, 2 * SPC], bf16, tag="oi")
                c1v = c1[:].rearrange("p (r w) -> p r w", w=W)
                t1v = t1[:].rearrange("p (r w) -> p r w", w=W)
                t2v = t2[:].rearrange("p (r w) -> p r w", w=W)
                oiv = oi[:].rearrange("p (r s w) -> p r s w", s=2, w=W)
                bias_c = bias_t[:, cc : cc + 1]
                nc.scalar.activation(
                    c1[:], ps[1], mybir.ActivationFunctionType.Identity,
                    bias=bias_c,
                )
                nc.vector.tensor_add(t1v, c1v, y2)
                nc.vector.tensor_add(oiv[:, :, 0, :], t1v, y0)
                nc.vector.tensor_sub(t2v, c1v, y2)
                nc.vector.tensor_sub(oiv[:, :, 1, :], t2v, y3)
                for r0, h2 in stores:
                    nc.scalar.dma_start(
                        out=y_d[b, och0 : och0 + ncha, 128 * h2 : 128 * h2 + 1024],
                        in_=oi[r0 : r0 + ncha, :],
                    )

            # weight loads staggered one sample ahead: b's weights at its
            # start, b+1's mid-way through b (after only h0's stores sit on
            # the scalar HWDGE ring), so x DMAs aren't starved at startup
            # and the ring never delays the next sample's weights much
            wviews = {}

            def load_weights(b):
                wts = []
                for kc in range(2):
                    wt = io_pool.tile([128, 12 * COUT], bf16, tag=f"w{b}_{kc}")
                    nc.scalar.dma_start(
                        out=wt[:], in_=wt_d[b, 128 * kc : 128 * kc + 128, :]
                    )
                    wts.append(wt)
                wp = io_pool.tile([128, 4 * COUT], bf16, tag=f"wp{b}")
                nc.scalar.dma_start(out=wp[:], in_=wp_d[b])
                wq = io_pool.tile([128, 4 * COUT], bf16, tag=f"wq{b}")
                nc.scalar.dma_start(out=wq[:], in_=wq_d[b])
                wviews[b] = (
                    [t[:].rearrange("p (i k c) -> p i k c", k=3, c=COUT) for t in wts],
                    wp[:].rearrange("p (i c) -> p i c", c=COUT),
                    wq[:].rearrange("p (i c) -> p i c", c=COUT),
                )

            load_weights(0)
            nc.scalar.dma_start(out=bias_t[:], in_=bias_d[:])

            # HAM warmup: the PE clock-gate only opens (1.2 -> 2.4 GHz)
            # after ~3.4us of sustained matmul activity, and re-throttles
            # after a ~3.4us idle window. The first real chains can't start
            # until ~14us (DMA latency + first x/w bytes + input transform),
            # and their early data-stalls kept resetting the warmup streak,
            # leaving the PE at half clock until ~31us. So: run dep-free
            # matmuls on scratch SBUF into a scratch PSUM bank, sized to end
            # right as the first real chain is ready. Results are never read.
            scratch = cpool.tile([128, 640], bf16, tag="warmup_src")
            nc.vector.memset(scratch[:], 0.0)
            warm_ps = pspool.tile([128, SPC], f32, tag="ps")
            # ~12 cold matmuls (~430ns each) reach the ~3.4-5us HAM window,
            # a few warm ones pad to ~15us; the gap until the first real
            # chain (~16us) is well under the ~3.4us MID re-throttle window
            NWARM = 18
            for wi in range(NWARM):
                nc.tensor.matmul(
                    warm_ps[:],
                    scratch[:, 512:640],
                    scratch[:, 0:512],
                    start=(wi == 0),
                    stop=(wi == NWARM - 1),
                )

            pending_drain = None

            for b in range(BLOC):
                wtv, wpv, wqv = wviews[b]

                for half in range(2):
                    if half == 1 and b + 1 < BLOC:
                        load_weights(b + 1)
                    base = half * HALF_BASE
                    xts = []
                    for kc in range(2):
                        xt = xpool.tile([128, HALF_IN], bf16, tag=f"x{kc}")
                        nc.sync.dma_start(
                            out=xt[:],
                            in_=xp_d[b, 128 * kc : 128 * kc + 128, base : base + HALF_IN],
                        )
                        xts.append(xt)
                    # ci tail chunk; upper 64 partitions hold a +1-column
                    # shifted copy (pairs kw taps in one K=128 step)
                    xt3 = xpool.tile([128, HALF_IN], bf16, tag="xt")
                    nc.sync.dma_start(
                        out=xt3[:64], in_=xp_d[b, 256:320, base : base + HALF_IN]
                    )
                    nc.sync.dma_start(
                        out=xt3[64:128, 0 : HALF_IN - 1],
                        in_=xp_d[b, 256:320, base + 1 : base + HALF_IN],
                    )

                    # input transform (SBUF-only), emitted position-major so
                    # the first chains gate on the first few ops, not all 12.
                    # DVE is ~3x faster per op than GpSimd, so DVE takes two
                    # of the three ci-chunks per position.
                    views = []
                    for xt in xts + [xt3]:
                        xq = xt[:].rearrange("p (q s w) -> p q s w", s=2, w=WP)
                        views.append((
                            xq[:, 0:RT, 0, :], xq[:, 0:RT, 1, :],
                            xq[:, 1 : RT + 1, 0, :], xq[:, 1 : RT + 1, 1, :],
                        ))
                    ms = [[None] * NPOS for _ in range(3)]
                    if b == 0 and half == 0:
                        # cold start: everything on DVE (3x faster than
                        # GpSimd), ordered by chain consumption so the first
                        # chains begin as soon as each x chunk lands
                        order = [(0, 0), (0, 1), (1, 0), (2, 0), (1, 1),
                                 (2, 1), (0, 2), (1, 2), (2, 2), (0, 3),
                                 (1, 3), (2, 3)]
                    else:
                        order = [(c, i) for i in range(NPOS) for c in range(3)]
                    for c, i in order:
                        a0, a1, a2, a3 = views[c]
                        mt = mpool.tile([128, MSP], bf16, tag=f"m{c}_{i}")
                        mv = mt[:].rearrange("p (r w) -> p r w", w=WP)
                        eng = nc.gpsimd if (c == 1 and not (b == 0 and half == 0)) else nc.vector
                        if i == 0:
                            eng.tensor_sub(mv, a0, a2)
                        elif i == 1:
                            eng.tensor_add(mv, a1, a2)
                        elif i == 2:
                            eng.tensor_sub(mv, a2, a1)
                        else:
                            eng.tensor_sub(mv, a1, a3)
                        ms[c][i] = mv
                    mt_full = ms[0], ms[1]
                    mt_tail = ms[2]

                    # previous half's last drain goes here, AFTER this half's
                    # transforms, so the DVE FIFO never head-of-line-blocks
                    # M production on the previous half's last PSUM stop
                    if pending_drain is not None:
                        emit_drain(*pending_drain)
                        pending_drain = None

                    def chain7(region, pos, glo, o0, osz):
                        """7 accumulating K-steps (start on first, no stop)."""
                        first = True
                        for kc in range(2):
                            for kw in range(3):
                                nc.tensor.matmul(
                                    region,
                                    wtv[kc][:, pos, kw, o0 : o0 + osz],
                                    mt_full[kc][pos][:, glo : glo + 8, kw : kw + W],
                                    start=first,
                                    stop=False,
                                )
                                first = False
                        nc.tensor.matmul(
                            region,
                            wpv[:, pos, o0 : o0 + osz],
                            mt_tail[pos][:, glo : glo + 8, 0:W],
                            start=False,
                            stop=False,
                        )

                    h2base = 16 * half

                    # groups: (cc0,g0), (cc1,g0), cc2(both g), (cc0,g1), (cc1,g1)
                    for cc, o0, g in ((0, 0, 0), (1, 128, 0), (2, 256, -1),
                                      (0, 0, 1), (1, 128, 1)):
                        if cc < 2:
                            glo = 8 * g
                            ps = [
                                pspool.tile([128, SPC], f32, tag="ps", name=f"ps{i}")
                                for i in range(NPOS)
                            ]
                            for pp in range(2):  # position pairs (0,1), (2,3)
                                pa, pb = 2 * pp, 2 * pp + 1
                                chain7(ps[pa][:], pa, glo, o0, 128)
                                chain7(ps[pb][:], pb, glo, o0, 128)
                                # kw2 K=64 steps, row-tiled across the pair
                                nc.tensor.matmul(
                                    ps[pa][:],
                                    wqv[0:64, pa, o0 : o0 + 128],
                                    mt_tail[pa][0:64, glo : glo + 8, 2 : 2 + W],
                                    start=False,
                                    stop=True,
                                )
                                nc.tensor.matmul(
                                    ps[pb][:],
                                    wqv[64:128, pb, o0 : o0 + 128],
                                    mt_tail[pb][64:128, glo : glo + 8, 1 : 1 + W],
                                    start=False,
                                    stop=True,
                                )
                                # cold start: the first groups' chains stall
                                # on x1/xt arrival; dep-free dummy matmuls
                                # absorb those holes so the HAM activity
                                # streak (and 2.4 GHz clock) is preserved
                                if b == 0 and half == 0 and g == 0:
                                    for wi in range(6):
                                        nc.tensor.matmul(
                                            warm_ps[:],
                                            scratch[:, 512:640],
                                            scratch[:, 0:512],
                                            start=(wi == 0),
                                            stop=(wi == 5),
                                        )
                            args = ([p[:] for p in ps], cc, o0, 128, b,
                                    [(0, h2base + glo)])
                            if (cc, g) == (1, 1):
                                pending_drain = args
                            else:
                                emit_drain(*args)
                        else:
                            # cc2: osz=64, col-tiled: r-group 0 in PSUM rows
                            # 0:64 / array cols 0:63, r-group 1 in rows 64:128
                            ps = [
                                pspool.tile([128, SPC], f32, tag="ps", name=f"pq{i}")
                                for i in range(NPOS)
                            ]
                            for pp in range(2):
                                pa, pb = 2 * pp, 2 * pp + 1
                                for p in (pa, pb):
                                    for sub in range(2):
                                        chain7(ps[p][64 * sub : 64 * sub + 64, :],
                                               p, 8 * sub, o0, 64)
                                # kw2 K=64: 4 matmuls in the 4 array quadrants
                                for sub in range(2):
                                    nc.tensor.matmul(
                                        ps[pa][64 * sub : 64 * sub + 64, :],
                                        wqv[0:64, pa, o0 : o0 + 64],
                                        mt_tail[pa][0:64, 8 * sub : 8 * sub + 8, 2 : 2 + W],
                                        start=False,
                                        stop=True,
                                    )
                                for sub in range(2):
                                    nc.tensor.matmul(
                                        ps[pb][64 * sub : 64 * sub + 64, :],
                                        wqv[64:128, pb, o0 : o0 + 64],
                                        mt_tail[pb][64:128, 8 * sub : 8 * sub + 8, 1 : 1 + W],
                                        start=False,
                                        stop=True,
                                    )
                            emit_drain([p[:] for p in ps], cc, o0, 64, b,
                                       [(0, h2base), (64, h2base + 8)])

            if pending_drain is not None:
                emit_drain(*pending_drain)

    nc.compile()
    return nc


def _get_nc():
    if "nc" not in _NC_CACHE:
        _NC_CACHE["nc"] = _build_nc()
    return _NC_CACHE["nc"]


def _prep_inputs(x, conv_w, conv_b, down_w, up_w, lora_id):
    bf = ml_dtypes.bfloat16
    x = np.asarray(x, dtype=np.float32)
    conv_w = np.asarray(conv_w, dtype=np.float32)
    conv_b = np.asarray(conv_b, dtype=np.float32)
    down_w = np.asarray(down_w, dtype=np.float32)
    up_w = np.asarray(up_w, dtype=np.float32)
    idx = np.asarray(lora_id).astype(np.int64) // LORA_STRIDE
    active = (idx >= 0).astype(np.float32)
    safe = np.clip(idx, 0, NUM_LORAS - 1)

    # Exact LoRA merge: W_lora[b,o,i,kh,kw] = sum_r up[o,r] down[r,i,kh,kw]
    lora = np.matmul(up_w[safe], down_w[safe].reshape(B, R, -1))
    lora = lora.reshape(B, COUT, CIN, 3, 3)
    weff = conv_w[None] + (SCALE * active)[:, None, None, None, None] * lora

    # Winograd F(2,3) weight transform over kh: [B, pos, co, ci, kw]
    g0, g1, g2 = weff[..., 0, :], weff[..., 1, :], weff[..., 2, :]
    Gg = np.stack([g0, 0.5 * (g0 + g1 + g2), 0.5 * (g0 - g1 + g2), g2], axis=1)
    # lhsT layout [b, ci, pos, kw, co]
    arr = np.ascontiguousarray(Gg.transpose(0, 3, 1, 4, 2))
    wt_main = arr[:, :256].reshape(B, 256, 12 * COUT).astype(bf)
    wp_all = np.empty((B, 128, NPOS, COUT), dtype=np.float32)
    wp_all[:, 0:64] = arr[:, 256:320, :, 0, :]
    wp_all[:, 64:128] = arr[:, 256:320, :, 1, :]
    wp_all = wp_all.reshape(B, 128, 4 * COUT).astype(bf)
    wq_all = np.empty((B, 128, NPOS, COUT), dtype=np.float32)
    wq_all[:, 0:64] = arr[:, 256:320, :, 2, :]
    wq_all[:, 64:128] = arr[:, 256:320, :, 2, :]
    wq_all = wq_all.reshape(B, 128, 4 * COUT).astype(bf)

    xp = np.pad(x, ((0, 0), (0, 0), (1, 1), (1, 1))).reshape(B, CIN, SP).astype(bf)
    bias2 = np.zeros((128, 3), dtype=np.float32)
    for cc, (o0, osz) in enumerate(CO_CHUNKS):
        bias2[:osz, cc] = conv_b[o0 : o0 + osz]
        if osz < 128:
            bias2[osz : 2 * osz, cc] = conv_b[o0 : o0 + osz]

    in_maps = [
        {
            "xp": np.ascontiguousarray(xp[c * BLOC : (c + 1) * BLOC]),
            "wt": np.ascontiguousarray(wt_main[c * BLOC : (c + 1) * BLOC]),
            "wp": np.ascontiguousarray(wp_all[c * BLOC : (c + 1) * BLOC]),
            "wq": np.ascontiguousarray(wq_all[c * BLOC : (c + 1) * BLOC]),
            "bias": bias2,
        }
        for c in range(NCORES)
    ]
    return in_maps


def run_device(in_maps, trace=False, tmpdir=None):
    from concourse.bass_utils import run_bass_kernel_spmd

    nc = _get_nc()
    return run_bass_kernel_spmd(
        nc, in_maps, list(range(NCORES)), trace=trace, tmpdir=tmpdir
    )


def kernel(x, conv_w, conv_b, down_w, up_w, lora_id):
    in_maps = _prep_inputs(x, conv_w, conv_b, down_w, up_w, lora_id)
    out = run_device(in_maps)
    y = np.concatenate(
        [np.asarray(out.results[c]["y"]) for c in range(NCORES)], axis=0
    )
    return np.ascontiguousarray(
        y.reshape(B, COUT, H, W).astype(np.float32)
    )
